# revision 45
# baseline (speedup 1.0000x reference)
"""Trainium2 Bass kernel for nn_BlockWithAttention (dense CNN block + attention).

Sharding: data-parallel over batch (B=16 -> 2 samples/core x 8 cores).

Scheduling design (tuned against the TimelineSim cost model):
- The PE pstate ramp model freezes each matmul's clock at cost time, so any
  PE idle gap poisons the following flood with 2-3.7x slower matmuls. Tiny
  "filler" matmuls ([128,32] ones, dedicated PSUM bank) bridge every
  potential idle window so real matmuls always cost at the 2.4GHz peak.
- BN batch-stat sync is chunked per 128-channel group (2 AllGathers per BN)
  and pipelined: BN1-ko0's collective flies during conv1-ko1, BN1-ko1's
  during conv2's ki0 phase, BN2's two during conv2-ki1/conv3-ki0. The
  collective launch (stats DMA + AllGather) and collect (readback on the
  ACT queue + reduce) are emitted separately so no engine queue is
  head-of-line blocked by a later chunk's collective.
- conv2/conv3 split into ki phases with 7+1 psums held open across the
  phase boundary (the 8th accumulator time-shares the filler PSUM bank).
- attention: V@P output matmuls are emitted before the softmax-denominator
  reduce; each block's denominator/output tail is deferred behind the next
  block's leading S matmuls so the PE never waits on the DVE add-tree;
  gamma is folded into wv/bv host-side; post-softmax tiles (P, V^T) are
  bf16 (errors ~0.4% on attention weights only).

All matmuls run in float32r (full PE rate at free-size>=256) except the
post-softmax bf16 pair; accumulation is fp32 in PSUM.
"""
import numpy as np

import concourse.bass as bass
import concourse.mybir as mybir
from concourse.bass_utils import run_bass_kernel_spmd
from concourse.tile import TileContext
from concourse.tile_rust import add_dep_helper

# ---- problem constants ----
B, C, H, W, T, CQ = 16, 256, 32, 32, 256, 32
NCORES = 8
BL = B // NCORES            # samples per core
KT = C // 128               # 128-channel tiles
HP, WP = H + 2, W + 2       # padded image
NPAD = HP * WP              # 1156
NPIX = B * H * W            # BN stat count (full batch)
N = H * W                   # 1024 spatial positions
RH = 16                     # rows per 512-px half
EPS = 1e-5
KCOLS = 9 * KT * 128        # weight cols per ko group (ko-major layout)

F32 = mybir.dt.float32
F32R = mybir.dt.float32r
BF16 = mybir.dt.bfloat16
AX = mybir.AxisListType
ALU = mybir.AluOpType
AF = mybir.ActivationFunctionType

U32 = mybir.dt.uint32
U16 = mybir.dt.uint16
ONE_F32_BITS = 0x3F800000

# conv1/conv2 operand dtype: bf16 halves DMA; set False to use f32r
USE_BF16 = False
CDT = BF16 if USE_BF16 else F32R

# filler-block sizes (tuned against TimelineSim)
FILL = {
    "ign0": 130,   # kernel start -> first conv1 matmul
    "A": 240,       # conv1/MLP -> conv2-ki0 (BN1-ko0 wait)
    "B": 15,       # conv2-ki0 -> conv2-ki1 (BN1-ko1 wait, insurance)
    "C": 300,      # conv2 end -> conv3-ki0 (BN2-ko0 collective wait)
    "D": 40,       # conv3-ki0 -> conv3-ki1 (insurance)
    "E": 25,       # conv3 -> attention V (y epilogue trail)
    "V": 20,       # per-sample V block lead-in
    "S": 20,       # per-(s,nh) S block lead-in
}

_wsplit_counter = [0]


def _split_packed_waits(nc, max_waits: int = 1):
    """The walrus build here rejects >1-2 packed sync-waits per instruction
    ("Too many sync wait commands"). Move excess waits onto standalone
    single-wait EventSemaphore carriers inserted before the instruction
    (same engine -> program order preserves gating)."""
    for f in nc.m.functions:
        for bb in f.blocks:
            il = bb.instructions
            i = 0
            while i < len(il):
                inst = il[i]
                si = inst.sync_info
                if si is not None and len(si.on_wait) > max_waits:
                    waits = list(si.on_wait)
                    movable = [w for w in waits if w.wait_reg is None]
                    fixed = [w for w in waits if w.wait_reg is not None]
                    keep_n = max(0, max_waits - len(fixed))
                    kept = fixed + movable[:keep_n]
                    move = movable[keep_n:]
                    if not move:
                        i += 1
                        continue
                    si.on_wait = kept
                    for w in move:
                        _wsplit_counter[0] += 1
                        ev = mybir.InstEventSemaphore(
                            name=f"I-wsplit-{_wsplit_counter[0]}",
                            opcode="EventSemaphore",
                            engine=inst.engine,
                            sync_info=mybir.SyncInfo(on_wait=[w], on_update=[]),
                        )
                        il.insert(i, ev)
                        i += 1
                i += 1


def _pad3(tile):
    return tile[:, :].rearrange("p (r c) -> p r c", c=WP)


def _interior(tile, r0=0, nr=H):
    return _pad3(tile)[:, 1 + r0:1 + r0 + nr, 1:1 + W]


def _tap(tile, dy, dx, r0, nr):
    return _pad3(tile)[:, r0 + dy:r0 + dy + nr, dx:dx + W]


def _memset_border(nc, tile):
    # gpsimd memset rejects float32r; write zero bits via an int bitcast of
    # matching width
    cast = U32 if mybir.dt.size(tile.dtype) == 4 else U16
    v = _pad3(tile)
    nc.gpsimd.memset(v[:, 0:1, :].bitcast(cast), 0)
    nc.gpsimd.memset(v[:, HP - 1:HP, :].bitcast(cast), 0)
    nc.gpsimd.memset(v[:, 1:HP - 1, 0:1].bitcast(cast), 0)
    nc.gpsimd.memset(v[:, 1:HP - 1, WP - 1:WP].bitcast(cast), 0)


def build(dt_conv=F32R, dt_attn=F32R, split: bool = True, fill=None):
    fill = dict(FILL, **(fill or {}))
    nc = bass.Bass(num_devices=NCORES)

    # ---- DRAM I/O ----
    xp_d = nc.dram_tensor("xp", [BL, KT, 128, NPAD], CDT, kind="ExternalInput")
    # ko-major conv weights: [ci][128(i), ((ko*9 + tap)*KT + ki)*128 + o]
    # conv1/conv2 in bf16 (errors laundered by the BNs); conv3 stays f32r so
    # the attention logits (exp-amplified) keep full precision.
    cwa_d = nc.dram_tensor("cwa", [2, 128, KT * KCOLS], CDT, kind="ExternalInput")
    cw2_d = nc.dram_tensor("cw2", [128, KT * KCOLS], F32R, kind="ExternalInput")
    w1t_d = nc.dram_tensor("w1t", [KT, 128, T], F32R, kind="ExternalInput")
    w2t_d = nc.dram_tensor("w2t", [KT, 128, C], F32R, kind="ExternalInput")
    consts_d = nc.dram_tensor("consts", [128, 22], F32R, kind="ExternalInput")
    wqt_d = nc.dram_tensor("wqt", [KT, 128, CQ], dt_attn, kind="ExternalInput")
    wkt_d = nc.dram_tensor("wkt", [KT, 128, CQ], dt_attn, kind="ExternalInput")
    wvt_d = nc.dram_tensor("wvt", [KT, 128, C], dt_attn, kind="ExternalInput")
    bq_d = nc.dram_tensor("bq", [CQ, 1], F32R, kind="ExternalInput")
    bk_d = nc.dram_tensor("bk", [CQ, 1], F32R, kind="ExternalInput")
    bv_d = nc.dram_tensor("bv", [1, C], dt_attn, kind="ExternalInput")
    out_d = nc.dram_tensor("out", [BL, KT, 128, N], F32R, kind="ExternalOutput")

    cc_in = [nc.dram_tensor(f"cc{i}_in", [128, 2], F32) for i in range(4)]
    cc_out = [nc.dram_tensor(f"cc{i}_out", [NCORES, 128, 2], F32,
                             addr_space="Shared") for i in range(4)]

    with TileContext(nc) as tc:
        with (
            tc.tile_pool(name="pconst", bufs=1) as pc,
            tc.tile_pool(name="pcw", bufs=2) as pcw,
            tc.tile_pool(name="ppad", bufs=8) as ppad,
            tc.tile_pool(name="py", bufs=4) as py,
            tc.tile_pool(name="psq", bufs=2) as psq,
            tc.tile_pool(name="pattn", bufs=1) as pat,
            tc.tile_pool(name="pstats", bufs=1) as pst,
            tc.tile_pool(name="ppsum", bufs=1, space="PSUM") as pps,
        ):
            def psum(nm):
                return pps.tile([128, 512], F32, tag="ps", bufs=7, name=nm)

            # ---- filler infrastructure: shared PSUM bank + ones tile.
            # The bank is time-shared with the 8th open conv psum (c2late/
            # c3late); each generation is a fresh pool tile on the same
            # 1-buf tag ----
            fones = pc.tile([128, 32], F32R, name="fones")
            nc.gpsimd.memset(fones[:, :].bitcast(U32), ONE_F32_BITS)
            fgen = [0]
            fps_cur = [None]

            def fill_bank_tile(shape, nm):
                fgen[0] += 1
                t = pps.tile(shape, F32, tag="fill", bufs=1,
                             name=f"{nm}{fgen[0]}")
                return t

            def filler(n):
                if n <= 0:
                    return
                if fps_cur[0] is None:
                    fps_cur[0] = fill_bank_tile([32, 512], "fps")
                fps = fps_cur[0]
                for _ in range(n):
                    nc.tensor.matmul(fps[0:32, 0:32], fones[:, :], fones[:, :],
                                     start=True, stop=True)

            # ---- big DMAs: x first, conv1 weights in ko-major halves ----
            cw_sb = [pcw.tile([128, KT * KCOLS], CDT if ci < 2 else F32R,
                              tag="cw", bufs=2, name=f"cw{ci}")
                     for ci in range(3)]
            x_pad = [[ppad.tile([128, NPAD], CDT, tag="xpad", bufs=4,
                                name=f"xp{s}{k}")
                      for k in range(KT)] for s in range(BL)]
            for k in range(KT):
                nc.sync.dma_start(out=x_pad[0][k][:, :], in_=xp_d[0, k, :, :])
            nc.sync.dma_start(out=cw_sb[0][:, 0:KCOLS], in_=cwa_d[0, :, 0:KCOLS])
            for k in range(KT):
                nc.sync.dma_start(out=x_pad[1][k][:, :], in_=xp_d[1, k, :, :])
            nc.sync.dma_start(out=cw_sb[0][:, KCOLS:2 * KCOLS],
                              in_=cwa_d[0, :, KCOLS:2 * KCOLS])

            # ---- persistent small tiles ----
            w1t_sb = [pc.tile([128, T], F32R, name=f"w1t{k}") for k in range(KT)]
            w2t_sb = [pc.tile([128, C], F32R, name=f"w2t{k}") for k in range(KT)]
            consts_sb = pc.tile([128, 22], F32R, name="consts_sb")

            def ccol(j, n=1):
                return consts_sb[:, j:j + n]

            cb_sb = [[ccol(ci * KT + k) for k in range(KT)] for ci in range(3)]
            bng_sb = [[ccol(6 + i * KT + k) for k in range(KT)] for i in range(2)]
            bnb_sb = [[ccol(10 + i * KT + k) for k in range(KT)] for i in range(2)]
            bt1_sb = [ccol(14 + k) for k in range(KT)]
            bt2_sb = [ccol(16 + k) for k in range(KT)]
            tt_sb = [ccol(18 + k * BL, BL) for k in range(KT)]
            wqt_sb = [pc.tile([128, CQ], dt_attn, name=f"wqt{k}") for k in range(KT)]
            wkt_sb = [pc.tile([128, CQ], dt_attn, name=f"wkt{k}") for k in range(KT)]
            wvt_sb = [pc.tile([128, C], dt_attn, name=f"wvt{k}") for k in range(KT)]
            bq_sb = pc.tile([CQ, 1], F32R, name="bq_sb")
            bk_sb = pc.tile([CQ, 1], F32R, name="bk_sb")
            bv_sb = pc.tile([1, C], dt_attn, name="bv_sb")
            ones_col = pc.tile([128, 1], BF16, name="ones_col")
            ones_row = pc.tile([1, 128], dt_attn, name="ones_row")

            nc.gpsimd.dma_start(out=consts_sb[:, :], in_=consts_d[:, :])
            for k in range(KT):
                nc.sync.dma_start(out=w1t_sb[k][:, :], in_=w1t_d[k, :, :])
                nc.sync.dma_start(out=w2t_sb[k][:, :], in_=w2t_d[k, :, :])
                nc.sync.dma_start(out=wqt_sb[k][:, :], in_=wqt_d[k, :, :])
                nc.sync.dma_start(out=wkt_sb[k][:, :], in_=wkt_d[k, :, :])
                nc.sync.dma_start(out=wvt_sb[k][:, :], in_=wvt_d[k, :, :])
            nc.gpsimd.dma_start(out=bq_sb[:, :], in_=bq_d[:, :])
            nc.gpsimd.dma_start(out=bk_sb[:, :], in_=bk_d[:, :])
            nc.gpsimd.dma_start(out=bv_sb[:, :], in_=bv_d[:, :])
            nc.gpsimd.memset(ones_col[:, :].bitcast(U16), 0x3F80)
            nc.gpsimd.memset(ones_row[:, :].bitcast(U32), ONE_F32_BITS)
            nc.sync.dma_start(out=cw_sb[1][:, :], in_=cwa_d[1, :, :])

            stats = [pst.tile([128, 16], F32, name=f"stats{i}") for i in range(2)]
            ccp = [pst.tile([128, 2], F32, name=f"ccp{i}") for i in range(4)]
            glob = [pst.tile([128, 2], F32, name=f"glob{i}") for i in range(4)]
            gall = [pst.tile([128, 2 * NCORES], F32, name=f"gall{i}")
                    for i in range(4)]
            for i in range(2):
                nc.gpsimd.memset(stats[i][:, :], 0.0)

            h1_pad = [[ppad.tile([128, NPAD], CDT, tag="h1pad", bufs=4,
                                 name=f"h1p{s}{k}")
                       for k in range(KT)] for s in range(BL)]
            h2_pad = [[ppad.tile([128, NPAD], F32R, tag="h2pad", bufs=4,
                                 name=f"h2p{s}{k}")
                       for k in range(KT)] for s in range(BL)]
            for s in range(BL):
                for k in range(KT):
                    _memset_border(nc, h1_pad[s][k])
                    _memset_border(nc, h2_pad[s][k])

            # ---- helpers ----
            def conv_part(ci, src_pads, s, ko, half, ki, ps3, first, last):
                r0 = half * RH
                for tap in range(9):
                    dy, dx = divmod(tap, 3)
                    j = (ko * 9 + tap) * KT + ki
                    nc.tensor.matmul(
                        ps3,
                        cw_sb[ci][:, j * 128:(j + 1) * 128],
                        _tap(src_pads[s][ki], dy, dx, r0, RH),
                        start=(first and tap == 0), stop=(last and tap == 8),
                    )

            def epilogue_stats(i, dst_pad, s, ko, half, ps3):
                col = ko * 4 + s * 2 + half
                nc.scalar.activation(
                    _interior(dst_pad[s][ko], half * RH, RH), ps3, AF.Relu,
                    bias=cb_sb[i][ko][:, :],
                    accum_out=stats[i][:, col:col + 1],
                )
                sq = psq.tile([128, 512], F32, tag="sq", name=f"sq{i}_{s}{ko}{half}")
                nc.scalar.activation(
                    sq[:, :].rearrange("p (r c) -> p r c", c=W),
                    _interior(dst_pad[s][ko], half * RH, RH), AF.Square,
                    accum_out=stats[i][:, 8 + col:9 + col],
                )

            cc_insts = {}

            def chunk_launch(i, ko):
                """reduce the (s,half) accumulator columns, ship + AllGather"""
                ci_ = i * KT + ko
                nc.vector.reduce_sum(
                    ccp[ci_][:, :],
                    stats[i][:, :].rearrange("p (k c) -> p k c", k=2)
                        [:, :, ko * 4:ko * 4 + 4], axis=AX.X)
                d1 = nc.sync.dma_start(out=cc_in[ci_][:, :], in_=ccp[ci_][:, :])
                cc = nc.gpsimd.collective_compute(
                    "AllGather", ALU.bypass,
                    replica_groups=[list(range(NCORES))],
                    ins=[cc_in[ci_][:].opt()], outs=[cc_out[ci_][:].opt()],
                )
                add_dep_helper(cc.ins, d1.ins, reason="cc waits on stats dma")
                cc_insts[ci_] = cc

            def chunk_collect(ci_):
                """readback (ACT queue: idle at collective-end times) + one
                reduce covering both the core and (s,half) axes. Emitted
                right before the bn_consts consumer so no engine queue gets
                head-of-line blocked by a later chunk's collective."""
                d2 = nc.scalar.dma_start(
                    out=gall[ci_][:, :],
                    in_=cc_out[ci_][:, :, :].rearrange("c p k -> p c k"))
                add_dep_helper(d2.ins, cc_insts[ci_].ins,
                               reason="readback waits on cc")
                nc.vector.reduce_sum(
                    glob[ci_][:, :],
                    gall[ci_][:, :].rearrange("p (c k) -> p k c", k=2), axis=AX.X)

            def bn_consts(ci_, i, ko):
                """returns (scale, NEGATED shift): norm = h*scl - nshf"""
                mm2 = pst.tile([128, 2], F32, name=f"mm2{ci_}")
                vb = pst.tile([128, 1], F32, name=f"vb{ci_}")
                rv = pst.tile([128, 1], F32, name=f"rv{ci_}")
                scl = pst.tile([128, 1], F32, name=f"scl{ci_}")
                nshf = pst.tile([128, 1], F32, name=f"nshf{ci_}")
                mean = mm2[:, 0:1]
                # [mean, E(x^2)] in one op
                nc.vector.tensor_scalar_mul(mm2[:, :], glob[ci_][:, :], 1.0 / NPIX)
                # vb = (mean*mean - ex2)*(-1) + EPS = var + eps
                nc.vector.scalar_tensor_tensor(
                    out=vb[:, :], in0=mean, scalar=mean, in1=mm2[:, 1:2],
                    op0=ALU.mult, op1=ALU.subtract)
                nc.vector.tensor_scalar(out=vb[:, :], in0=vb[:, :], scalar1=-1.0,
                                        scalar2=EPS, op0=ALU.mult, op1=ALU.add)
                nc.vector.reciprocal(rv[:, :], vb[:, :])
                nc.scalar.activation(rv[:, :], rv[:, :], AF.Sqrt)
                # one Newton step for full fp32 accuracy:
                # y' = y*(1.5 - 0.5*v*y*y)
                t1 = pst.tile([128, 1], F32, name=f"nr{ci_}")
                nc.vector.scalar_tensor_tensor(
                    out=t1[:, :], in0=rv[:, :], scalar=rv[:, :], in1=vb[:, :],
                    op0=ALU.mult, op1=ALU.mult)
                nc.vector.tensor_scalar(out=t1[:, :], in0=t1[:, :], scalar1=-0.5,
                                        scalar2=1.5, op0=ALU.mult, op1=ALU.add)
                nc.vector.tensor_tensor(rv[:, :], rv[:, :], t1[:, :], ALU.mult)
                nc.vector.tensor_tensor(scl[:, :], rv[:, :], bng_sb[i][ko][:, :],
                                        ALU.mult)
                # nshf = mean*scl - beta  (norm applies h*scl - nshf)
                nc.vector.scalar_tensor_tensor(
                    out=nshf[:, :], in0=mean, scalar=scl[:, :],
                    in1=bnb_sb[i][ko][:, :], op0=ALU.mult, op1=ALU.subtract)
                return scl, nshf

            # ================= ignition + conv1 (chunked by ko) ============
            filler(fill["ign0"])
            for ko in range(KT):
                for s in range(BL):
                    for half in range(2):
                        ps = psum(f"c1_{s}{ko}{half}")
                        ps3 = ps[:, :].rearrange("p (r c) -> p r c", c=W)
                        for ki in range(KT):
                            conv_part(0, x_pad, s, ko, half, ki, ps3,
                                      first=(ki == 0), last=(ki == KT - 1))
                        epilogue_stats(0, h1_pad, s, ko, half, ps3)
                chunk_launch(0, ko)
                if ko == 1:
                    # conv3 weights last: they reuse conv1's weight buffer
                    # (ring) AND must not delay the BN1 stats transfers on
                    # the shared DMA engines
                    nc.sync.dma_start(out=cw_sb[2][:, 0:KCOLS],
                                      in_=cw2_d[:, 0:KCOLS])
                    nc.sync.dma_start(out=cw_sb[2][:, KCOLS:2 * KCOLS],
                                      in_=cw2_d[:, KCOLS:2 * KCOLS])

            # ---- time MLP ----
            te1_sb = [pst.tile([128, BL], F32R, name=f"te1_{m}") for m in range(KT)]
            te_sb = [pst.tile([128, BL], F32R, name=f"te_{m}") for m in range(KT)]
            for mo in range(KT):
                ps = psum(f"mlp1_{mo}")
                for ki in range(KT):
                    nc.tensor.matmul(ps[:, 0:BL],
                                     w1t_sb[ki][:, mo * 128:(mo + 1) * 128],
                                     tt_sb[ki][:, :],
                                     start=(ki == 0), stop=(ki == KT - 1))
                nc.scalar.activation(te1_sb[mo][:, :], ps[:, 0:BL], AF.Relu,
                                     bias=bt1_sb[mo][:, :])
            for mo in range(KT):
                ps = psum(f"mlp2_{mo}")
                for ki in range(KT):
                    nc.tensor.matmul(ps[:, 0:BL],
                                     w2t_sb[ki][:, mo * 128:(mo + 1) * 128],
                                     te1_sb[ki][:, :],
                                     start=(ki == 0), stop=(ki == KT - 1))
                nc.scalar.activation(te_sb[mo][:, :], ps[:, 0:BL], AF.Relu,
                                     bias=bt2_sb[mo][:, :])

            # ================= BN1-ko0 -> conv2 ki0 phase ==================
            chunk_collect(0)
            scl0, shf0 = bn_consts(0, 0, 0)
            for s in range(BL):
                bsk = pst.tile([128, 1], F32, name=f"b1s{s}0")
                nc.vector.tensor_tensor(bsk[:, :], shf0[:, :],
                                        te_sb[0][:, s:s + 1], ALU.subtract)
                eng = nc.vector if s == 0 else nc.gpsimd
                eng.tensor_scalar(out=_interior(h1_pad[s][0]),
                                  in0=_interior(h1_pad[s][0]),
                                  scalar1=scl0[:, :], scalar2=bsk[:, :],
                                  op0=ALU.mult, op1=ALU.subtract)

            filler(fill["A"])
            # 7 pool psums + the filler bank hold all 8 accumulators open
            # across the ki-phase boundary
            c2keys = [(s, ko, half) for s in range(BL) for ko in range(KT)
                      for half in range(2)]
            c2ps = {}
            fps_cur[0] = None
            for i_, (s, ko, half) in enumerate(c2keys):
                if i_ == len(c2keys) - 1:
                    ps = fill_bank_tile([128, 512], "c2late")
                else:
                    ps = psum(f"c2_{s}{ko}{half}")
                c2ps[(s, ko, half)] = ps
                ps3 = ps[:, :].rearrange("p (r c) -> p r c", c=W)
                conv_part(1, h1_pad, s, ko, half, 0, ps3, first=True, last=False)

            # ---- BN1-ko1 -> normalize -> conv2 ki1 (ko-ordered) ----
            chunk_collect(1)
            scl1, shf1 = bn_consts(1, 0, 1)
            for s in range(BL):
                bsk = pst.tile([128, 1], F32, name=f"b1s{s}1")
                nc.vector.tensor_tensor(bsk[:, :], shf1[:, :],
                                        te_sb[1][:, s:s + 1], ALU.subtract)
                eng = nc.vector if s == 0 else nc.gpsimd
                eng.tensor_scalar(out=_interior(h1_pad[s][1]),
                                  in0=_interior(h1_pad[s][1]),
                                  scalar1=scl1[:, :], scalar2=bsk[:, :],
                                  op0=ALU.mult, op1=ALU.subtract)

            for ko in range(KT):
                for (s, ko_, half) in c2keys:
                    if ko_ != ko:
                        continue
                    ps = c2ps[(s, ko_, half)]
                    ps3 = ps[:, :].rearrange("p (r c) -> p r c", c=W)
                    conv_part(1, h1_pad, s, ko_, half, 1, ps3,
                              first=False, last=True)
                    epilogue_stats(1, h2_pad, s, ko_, half, ps3)
                chunk_launch(1, ko)

            # ================= BN2-ko0 -> conv3 ki0 phase ==================
            chunk_collect(2)
            scl2, shf2 = bn_consts(2, 1, 0)
            with nc.allow_low_precision(reason="f32r==f32 bit layout"):
                for s in range(BL):
                    eng = nc.vector if s == 0 else nc.gpsimd
                    eng.tensor_scalar(out=_interior(h2_pad[s][0]),
                                      in0=_interior(h2_pad[s][0]),
                                      scalar1=scl2[:, :], scalar2=shf2[:, :],
                                      op0=ALU.mult, op1=ALU.subtract)

            filler(fill["C"])
            c3ps = {}
            fps_cur[0] = None
            for i_, (s, ko, half) in enumerate(c2keys):
                if i_ == len(c2keys) - 1:
                    ps = fill_bank_tile([128, 512], "c3late")
                else:
                    ps = psum(f"c3_{s}{ko}{half}")
                c3ps[(s, ko, half)] = ps
                ps3 = ps[:, :].rearrange("p (r c) -> p r c", c=W)
                conv_part(2, h2_pad, s, ko, half, 0, ps3, first=True, last=False)

            chunk_collect(3)
            scl3, shf3 = bn_consts(3, 1, 1)
            with nc.allow_low_precision(reason="f32r==f32 bit layout"):
                for s in range(BL):
                    eng = nc.vector if s == 0 else nc.gpsimd
                    eng.tensor_scalar(out=_interior(h2_pad[s][1]),
                                      in0=_interior(h2_pad[s][1]),
                                      scalar1=scl3[:, :], scalar2=shf3[:, :],
                                      op0=ALU.mult, op1=ALU.subtract)

            y_sb = [[py.tile([128, N], dt_attn, tag="y", name=f"y{s}{k}")
                     for k in range(KT)] for s in range(BL)]
            for (s, ko, half) in c2keys:
                ps = c3ps[(s, ko, half)]
                ps3 = ps[:, :].rearrange("p (r c) -> p r c", c=W)
                conv_part(2, h2_pad, s, ko, half, 1, ps3,
                          first=False, last=True)
                nc.scalar.activation(
                    y_sb[s][ko][:, half * 512:(half + 1) * 512],
                    ps[:, :], AF.Identity, bias=cb_sb[2][ko][:, :])

            # ================= attention (per sample) ======================
            fps_cur[0] = None
            filler(fill["E"])

            def emit_v_group(s, vt, y_s, nt):
                ps = psum(f"vps{s}{nt}")
                pv = ps[:, 0:C]
                for c2 in range(KT):
                    nc.tensor.matmul(pv, y_s[c2][:, nt * 128:(nt + 1) * 128],
                                     wvt_sb[c2][:, :], start=(c2 == 0), stop=False)
                nc.tensor.matmul(pv, ones_row[:, :], bv_sb[:, :],
                                 start=False, stop=True)
                v = pat.tile([128, C], BF16, tag="vt", bufs=9, name=f"vt{s}{nt}")
                if nt % 2 == 0:
                    nc.vector.tensor_copy(v[:, :], pv)
                else:
                    nc.scalar.copy(v[:, :], pv)
                vt.append(v)

            def emit_s_exp(st, s, nh, mt, k_sb, q_sb, ptiles):
                nc.tensor.matmul(st[mt][:, :], k_sb[:, mt * 128:(mt + 1) * 128],
                                 q_sb[:, nh * 512:(nh + 1) * 512],
                                 start=True, stop=True)
                p = pat.tile([128, 512], BF16, tag="P", bufs=12,
                             name=f"P{s}{nh}{mt}")
                nc.scalar.activation(p[:, :], st[mt][:, :], AF.Exp)
                ptiles.append(p)

            def emit_pr_pacc(s, nh, vt, ptiles, pr_ps):
                for c2 in range(KT):
                    for mt in range(8):
                        nc.tensor.matmul(pr_ps[c2][:, :],
                                         vt[mt][:, c2 * 128:(c2 + 1) * 128],
                                         ptiles[mt][:, :],
                                         start=(mt == 0), stop=(mt == 7))
                pacc = [pat.tile([128, 512], BF16, tag="pacc", bufs=4,
                                 name=f"pacc{s}{nh}{h}") for h in range(2)]
                for h in range(2):
                    nc.vector.tensor_tensor(pacc[h][:, :],
                                            ptiles[4 * h][:, :],
                                            ptiles[4 * h + 1][:, :], ALU.add)
                    nc.vector.tensor_tensor(pacc[h][:, :], pacc[h][:, :],
                                            ptiles[4 * h + 2][:, :], ALU.add)
                    nc.vector.tensor_tensor(pacc[h][:, :], pacc[h][:, :],
                                            ptiles[4 * h + 3][:, :], ALU.add)
                return pacc

            def emit_tail1(s, nh, pd, pacc):
                """denominator matmuls + reciprocal (part 1)"""
                nc.tensor.matmul(pd[0:1, :], ones_col[:, :], pacc[0][:, :],
                                 start=True, stop=False)
                nc.tensor.matmul(pd[0:1, :], ones_col[:, :], pacc[1][:, :],
                                 start=False, stop=True)
                rcp = pat.tile([1, 512], dt_attn, tag="rcp", bufs=2,
                               name=f"rcp{s}{nh}")
                with nc.allow_low_precision(reason="f32r==f32 bit layout"):
                    nc.vector.reciprocal(rcp[:, :], pd[0:1, :])
                return rcp

            def emit_tail2(s, nh, pb, rcp, pr_ps, res_t, y_s, last_blk):
                """broadcast + scale + residual + output DMA (part 2)"""
                nc.tensor.matmul(pb[:, :], ones_row[:, :], rcp[:, :],
                                 start=True, stop=True)
                rbt = pat.tile([128, 512], F32, tag="rb", bufs=2,
                               name=f"rb{s}{nh}")
                nc.vector.tensor_copy(rbt[:, :], pb[:, :])
                # out = (V @ P) / denom + y  (gamma folded into wv/bv on host)
                for c2 in range(KT):
                    rs = res_t[:, c2 * N + nh * 512:c2 * N + (nh + 1) * 512]
                    nc.vector.tensor_tensor(rs, pr_ps[c2][:, :], rbt[:, :],
                                            ALU.mult)
                    eng = nc.vector if last_blk else nc.gpsimd
                    with nc.allow_low_precision(reason="f32r==f32 bits"):
                        eng.tensor_tensor(rs, rs,
                                          y_s[c2][:, nh * 512:(nh + 1) * 512],
                                          ALU.add)
                    if last_blk:
                        nc.sync.dma_start(
                            out=out_d[s, c2, :, nh * 512:(nh + 1) * 512],
                            in_=rs)
                if not last_blk:
                    nc.sync.dma_start(
                        out=out_d[s, :, :, nh * 512:(nh + 1) * 512]
                            .rearrange("k p n -> p k n"),
                        in_=res_t[:, :].rearrange("p (k n) -> p k n", k=KT)
                            [:, :, nh * 512:(nh + 1) * 512])

            deferred = [None]
            for s in range(BL):
                vt = []
                filler(fill["V"])
                y_s = y_sb[s]
                for nt in range(8):
                    emit_v_group(s, vt, y_s, nt)
                    if deferred[0] is not None and nt == 2 \
                            and len(deferred[0]) == 7:
                        ds, dpd, dpb, dpacc, dpr, dres, dys = deferred[0]
                        deferred[0] = (ds, dpb, emit_tail1(ds, 1, dpd, dpacc),
                                       dpr, dres, dys)
                    elif deferred[0] is not None and nt == 4 \
                            and len(deferred[0]) == 6:
                        ds, dpb, drcp, dpr, dres, dys = deferred[0]
                        emit_tail2(ds, 1, dpb, drcp, dpr, dres, dys,
                                   last_blk=False)
                        deferred[0] = None

                q_sb = pat.tile([CQ, N], dt_attn, tag="q", bufs=2, name=f"q{s}")
                k_sb = pat.tile([CQ, N], dt_attn, tag="k", bufs=2, name=f"k{s}")
                for nh in range(2):
                    psq_ = psum(f"qps{s}{nh}")
                    for c2 in range(KT):
                        nc.tensor.matmul(psq_[0:CQ, :], wqt_sb[c2][:, :],
                                         y_s[c2][:, nh * 512:(nh + 1) * 512],
                                         start=(c2 == 0), stop=(c2 == KT - 1))
                    nc.scalar.activation(q_sb[:, nh * 512:(nh + 1) * 512],
                                         psq_[0:CQ, :], AF.Identity, bias=bq_sb[:, :])
                    psk_ = psum(f"kps{s}{nh}")
                    for c2 in range(KT):
                        nc.tensor.matmul(psk_[0:CQ, :], wkt_sb[c2][:, :],
                                         y_s[c2][:, nh * 512:(nh + 1) * 512],
                                         start=(c2 == 0), stop=(c2 == KT - 1))
                    nc.scalar.activation(k_sb[:, nh * 512:(nh + 1) * 512],
                                         psk_[0:CQ, :], AF.Identity, bias=bk_sb[:, :])

                res_t = pat.tile([128, KT * N], F32R, tag="res", bufs=2,
                                 name=f"res{s}")
                # ---- nh0 block, tail deferred behind nh1's first S group ----
                st0 = [psum(f"sps{s}0{mt}") for mt in range(7)]
                st0.append(fill_bank_tile([128, 512], f"sps{s}0x"))
                fps_cur[0] = None
                pt0 = []
                for mt in range(8):
                    emit_s_exp(st0, s, 0, mt, k_sb, q_sb, pt0)
                # nh1's first 3 S psums + nh0's pd/pb claim ring slots before
                # the pr psums, so later S matmuls never wait on a pr drain
                st1 = [psum(f"sps{s}1{mt}") for mt in range(3)]
                pd0 = psum(f"dps{s}0")
                pb0 = psum(f"bps{s}0")
                pr0 = [psum(f"rps{s}0{c2}") for c2 in range(KT)]
                pacc0 = emit_pr_pacc(s, 0, vt, pt0, pr0)
                pt1 = []
                for mt in range(3):
                    emit_s_exp(st1, s, 1, mt, k_sb, q_sb, pt1)
                rcp0 = emit_tail1(s, 0, pd0, pacc0)
                # ---- nh1 block (nh0's tail part 2 lands mid-S) ----
                for mt in range(3, 7):
                    st1.append(psum(f"sps{s}1{mt}"))
                st1.append(fill_bank_tile([128, 512], f"sps{s}1x"))
                fps_cur[0] = None
                for mt in range(3, 8):
                    emit_s_exp(st1, s, 1, mt, k_sb, q_sb, pt1)
                    if mt == 4:
                        emit_tail2(s, 0, pb0, rcp0, pr0, res_t, y_s,
                                   last_blk=False)
                pd1 = psum(f"dps{s}1")
                pb1 = psum(f"bps{s}1")
                pr1 = [psum(f"rps{s}1{c2}") for c2 in range(KT)]
                pacc1 = emit_pr_pacc(s, 1, vt, pt1, pr1)
                if s == BL - 1:
                    rcp1 = emit_tail1(s, 1, pd1, pacc1)
                    emit_tail2(s, 1, pb1, rcp1, pr1, res_t, y_s,
                               last_blk=True)
                else:
                    deferred[0] = (s, pd1, pb1, pacc1, pr1, res_t, y_s)

    if split:
        _split_packed_waits(nc)
    return nc


def _prep_inputs(inputs):
    import ml_dtypes
    bf16 = ml_dtypes.bfloat16 if USE_BF16 else np.float32
    f32 = np.float32
    x = np.asarray(inputs["x"], f32)
    t = np.asarray(inputs["t"], f32)

    def conv_w(w):
        w6 = np.asarray(w, f32).reshape(KT, 128, KT, 128, 3, 3)  # ko,o,ki,i,dy,dx
        arr = w6.transpose(3, 0, 4, 5, 2, 1)  # i,ko,dy,dx,ki,o
        return np.ascontiguousarray(arr.reshape(128, KT * KCOLS))

    cwa = np.stack([conv_w(inputs["w_c1"]),
                    conv_w(inputs["w_c2"])]).astype(bf16)
    cw2 = conv_w(inputs["w_tr"])
    w1t = np.ascontiguousarray(np.asarray(inputs["w_t1"], f32).T.reshape(KT, 128, T))
    w2t = np.ascontiguousarray(np.asarray(inputs["w_t2"], f32).T.reshape(KT, 128, C))
    consts = np.zeros((128, 22), f32)
    for ci, k2 in enumerate(("b_c1", "b_c2", "b_tr")):
        consts[:, ci * KT:(ci + 1) * KT] = np.asarray(inputs[k2], f32).reshape(KT, 128).T
    for i, (gk, bk2) in enumerate((("bn1_g", "bn1_b"), ("bn2_g", "bn2_b"))):
        consts[:, 6 + i * KT:6 + (i + 1) * KT] = np.asarray(inputs[gk], f32).reshape(KT, 128).T
        consts[:, 10 + i * KT:10 + (i + 1) * KT] = np.asarray(inputs[bk2], f32).reshape(KT, 128).T
    consts[:, 14:16] = np.asarray(inputs["b_t1"], f32).reshape(KT, 128).T
    consts[:, 16:18] = np.asarray(inputs["b_t2"], f32).reshape(KT, 128).T
    gam = float(np.asarray(inputs["gamma"], f32).reshape(()))
    wqt = np.ascontiguousarray(np.asarray(inputs["wq"], f32).T.reshape(KT, 128, CQ))
    wkt = np.ascontiguousarray(np.asarray(inputs["wk"], f32).T.reshape(KT, 128, CQ))
    # gamma folded into V projection (out = gamma*out_att + y)
    wvt = np.ascontiguousarray(
        (gam * np.asarray(inputs["wv"], f32)).T.reshape(KT, 128, C))
    bq = np.asarray(inputs["bq"], f32).reshape(CQ, 1)
    bk = np.asarray(inputs["bk"], f32).reshape(CQ, 1)
    bv = (gam * np.asarray(inputs["bv"], f32)).reshape(1, C)

    xp = np.zeros((B, KT, 128, HP, WP), bf16)
    xp[:, :, :, 1:1 + H, 1:1 + W] = x.reshape(B, KT, 128, H, W).astype(bf16)
    xp = xp.reshape(B, KT, 128, NPAD)
    ttr = np.ascontiguousarray(t.T.reshape(KT, 128, B))

    shared = dict(cwa=cwa, cw2=cw2, w1t=w1t, w2t=w2t,
                  wqt=wqt, wkt=wkt, wvt=wvt, bq=bq, bk=bk, bv=bv)
    per_core = []
    for c in range(NCORES):
        m = dict(shared)
        m["xp"] = np.ascontiguousarray(xp[c * BL:(c + 1) * BL])
        cc_consts = consts.copy()
        for k in range(KT):
            cc_consts[:, 18 + k * BL:18 + (k + 1) * BL] = \
                ttr[k, :, c * BL:(c + 1) * BL]
        m["consts"] = cc_consts
        per_core.append(m)
    return per_core


def _unshard(results):
    out = np.empty((B, C, H, W), np.float32)
    for c in range(NCORES):
        o = results[c]["out"].reshape(BL, KT, 128, H, W)
        for s in range(BL):
            out[c * BL + s] = o[s].reshape(C, H, W)
    return out


_cache = {}

DT_CONV = F32R
DT_ATTN = F32R


def kernel(**inputs) -> np.ndarray:
    key = ("nc", str(DT_CONV), str(DT_ATTN))
    if key not in _cache:
        _cache[key] = build(dt_conv=DT_CONV, dt_attn=DT_ATTN)
    nc = _cache[key]
    per_core = _prep_inputs(inputs)
    try:
        res = run_bass_kernel_spmd(nc, per_core, core_ids=list(range(NCORES)))
    except Exception:
        # transient NRT_EXEC_UNIT_UNRECOVERABLE errors recover on re-execute
        res = run_bass_kernel_spmd(nc, per_core, core_ids=list(range(NCORES)))
    return _unshard(res.results)
